# revision 28
# baseline (speedup 1.0000x reference)
"""Class-conditional BatchNorm2d (eval path, alpha=0.5) on 8 Trainium2 cores.

Strategy (data-parallel over batch, per the sharding hint):
  - Each of the 8 cores gets 16 of the 128 samples; the small stat tables
    (global/class running mean/var, weight, bias) are replicated.
  - On-device, per core:
      1. one-hot(labels) built with iota + is_equal, transposed [class, b]
      2. PE matmul gathers class stats:  meanT[c,b] = class_mean[labels[b], c]
      3. interpolate with global stats, sqrt+reciprocal -> inv_std
      4. scaleT[c,b] = inv_std*weight[c]; shiftT[c,b] = bias[c]-mean*scale
      5. stream each sample [128ch x 3136px] through one fused
         tensor_scalar (x*scale + shift) with per-partition scalars.
  - Memory-bound: 49 MiB HBM traffic per core (load + store), compute hides
    underneath the DMA.
"""

import numpy as np
from contextlib import ExitStack

import concourse.bacc as bacc
import concourse.tile as tile
from concourse import mybir
from concourse.bass_utils import run_bass_kernel_spmd

B, C, H, W = 128, 128, 56, 56
HW = H * W
NCORES = 8
BS = B // NCORES  # 16 samples per core
NCLS = 100
EPS = 1e-5
ALPHA = 0.5

F32 = mybir.dt.float32
I32 = mybir.dt.int32

_CACHED_NC = None


def _build_nc():
    nc = bacc.Bacc(
        "TRN2",
        debug=False,
        enable_asserts=False,
        target_bir_lowering=False,
        num_devices=NCORES,
    )

    x_d = nc.dram_tensor("x", [BS, C, HW], F32, kind="ExternalInput")
    lbl_d = nc.dram_tensor("labels", [1, BS], I32, kind="ExternalInput")
    # packed [weight | bias] columns — one DMA
    cols_d = nc.dram_tensor("cols", [C, 2], F32, kind="ExternalInput")
    # packed [class_mean | class_var] along free dim, plus row NCLS =
    # [gmean | gvar] so the alpha-interpolation folds into the gather
    # matmul — one DMA
    cstats_d = nc.dram_tensor(
        "cstats", [NCLS + 1, 2 * C], F32, kind="ExternalInput"
    )
    out_d = nc.dram_tensor("out", [BS, C, HW], F32, kind="ExternalOutput")

    with tile.TileContext(nc) as tc, ExitStack() as ctx:
        const = ctx.enter_context(tc.tile_pool(name="const", bufs=1))
        psum = ctx.enter_context(tc.tile_pool(name="psum", bufs=1, space="PSUM"))
        data = ctx.enter_context(tc.tile_pool(name="data", bufs=8))

        # ---- small tables (head of the sync ring: ~2us of sequencer
        # time before the big loads, but the stat chain finishes before
        # the first load lands, so the first stores never stall the
        # ring) ----
        cstats_sb = const.tile([NCLS + 1, 2 * C], F32)
        nc.sync.dma_start(cstats_sb[:], cstats_d.ap())
        cols_sb = const.tile([C, 2], F32)
        nc.sync.dma_start(cols_sb[:], cols_d.ap())
        lbl_i = const.tile([1, BS], I32)
        nc.sync.dma_start(lbl_i[:], lbl_d.ap())
        w_col = cols_sb[:, 0:1]
        b_col = cols_sb[:, 1:2]

        # labels -> f32
        lbl_f = const.tile([1, BS], F32)
        nc.vector.tensor_copy(lbl_f[:], lbl_i[:])

        # broadcast labels across all 128 partitions via a K=1 matmul
        ones_row = const.tile([1, C], F32)
        nc.vector.memset(ones_row[:], 1.0)
        lbl_bc = psum.tile([C, BS], F32)
        nc.tensor.matmul(lbl_bc[:], ones_row[:], lbl_f[:], start=True, stop=True)

        # R[k, b] = alpha * (labels[b] == k-1) for k>=1, row 0 = 1-alpha:
        # the gather matmul against the stats table (row 0 = global,
        # rows 1..100 = per-class) directly yields
        # alpha*class_stat[labels[b]] + (1-alpha)*global_stat.
        # (Global row sits at partition 0 because SBUF partition slices
        # must start on 0/32/64/96.)
        iota_i = const.tile([C, 1], I32)
        nc.gpsimd.iota(iota_i[:], pattern=[[0, 1]], base=-1, channel_multiplier=1)
        iota_f = const.tile([C, 1], F32)
        nc.vector.tensor_copy(iota_f[:], iota_i[:])
        onehotT = const.tile([C, BS], F32)
        nc.vector.tensor_scalar(
            onehotT[:], lbl_bc[:], iota_f[:], ALPHA,
            mybir.AluOpType.is_equal, mybir.AluOpType.mult,
        )
        nc.vector.memset(onehotT[0:1, :], 1.0 - ALPHA)

        # gather + interpolate in one matmul per stat
        meanT = psum.tile([C, BS], F32)
        nc.tensor.matmul(
            meanT[:], cstats_sb[:, 0:C], onehotT[: NCLS + 1, :],
            start=True, stop=True,
        )
        varT = psum.tile([C, BS], F32)
        nc.tensor.matmul(
            varT[:], cstats_sb[:, C : 2 * C], onehotT[: NCLS + 1, :],
            start=True, stop=True,
        )

        # inv_std = 1/sqrt(var + eps)
        eps_col = const.tile([C, 1], F32)
        nc.vector.memset(eps_col[:], EPS)
        stdT = const.tile([C, BS], F32)
        nc.scalar.activation(
            stdT[:], varT[:], mybir.ActivationFunctionType.Sqrt, bias=eps_col[:]
        )
        invT = const.tile([C, BS], F32)
        nc.vector.reciprocal(invT[:], stdT[:])

        # scale = inv_std * weight ; shift = bias - mean * scale
        scaleT = const.tile([C, BS], F32)
        nc.vector.tensor_scalar(
            scaleT[:], invT[:], w_col, None, mybir.AluOpType.mult
        )
        msc = const.tile([C, BS], F32)
        nc.vector.tensor_tensor(msc[:], meanT[:], scaleT[:], mybir.AluOpType.mult)
        shiftT = const.tile([C, BS], F32)
        nc.vector.tensor_scalar(
            shiftT[:], msc[:], -1.0, b_col,
            mybir.AluOpType.mult, mybir.AluOpType.add,
        )

        # ---- stream the samples: out = x*scale + shift ----
        # One HWDGE ring (sync) carries all big transfers; deep bufs let
        # Tile front-load loads so every store's wait is pre-satisfied
        # when the in-order sequencer reaches it. 1-sample [128, 3136]
        # tiles keep per-partition runs contiguous (the efficient DMA
        # descriptor shape — 3D/transposed APs measured ~13% slower).
        for i in range(BS):
            xt = data.tile([C, HW], F32)
            nc.sync.dma_start(xt[:], x_d.ap()[i])
            nc.vector.tensor_scalar(
                xt[:], xt[:], scaleT[:, i : i + 1], shiftT[:, i : i + 1],
                mybir.AluOpType.mult, mybir.AluOpType.add,
            )
            nc.sync.dma_start(out_d.ap()[i], xt[:])

    nc.compile()
    return nc


def _get_nc():
    global _CACHED_NC
    if _CACHED_NC is None:
        _CACHED_NC = _build_nc()
    return _CACHED_NC


def _make_in_maps(inputs):
    x = np.ascontiguousarray(np.asarray(inputs["x"], dtype=np.float32)).reshape(
        B, C, HW
    )
    labels = np.asarray(inputs["labels"]).astype(np.int32)
    cols = np.ascontiguousarray(
        np.stack(
            [
                np.asarray(inputs["weight"], dtype=np.float32),
                np.asarray(inputs["bias"], dtype=np.float32),
            ],
            axis=1,
        )
    )
    cstats_cls = np.concatenate(
        [
            np.asarray(inputs["class_running_mean"], dtype=np.float32),
            np.asarray(inputs["class_running_var"], dtype=np.float32),
        ],
        axis=1,
    )
    grow = np.concatenate(
        [
            np.asarray(inputs["global_running_mean"], dtype=np.float32),
            np.asarray(inputs["global_running_var"], dtype=np.float32),
        ]
    )[None, :]
    cstats = np.ascontiguousarray(np.vstack([grow, cstats_cls]))

    in_maps = []
    for i in range(NCORES):
        sl = slice(i * BS, (i + 1) * BS)
        in_maps.append(
            {
                "x": np.ascontiguousarray(x[sl]),
                "labels": np.ascontiguousarray(labels[sl]).reshape(1, BS),
                "cols": cols,
                "cstats": cstats,
            }
        )
    return in_maps


def _run(inputs, trace=False, **kwargs):
    nc = _get_nc()
    in_maps = _make_in_maps(inputs)
    return run_bass_kernel_spmd(
        nc, in_maps, list(range(NCORES)), trace=trace, **kwargs
    )


def kernel(**inputs) -> np.ndarray:
    res = _run(inputs, trace=False)
    out = np.empty((B, C, H, W), dtype=np.float32)
    for i in range(NCORES):
        out[i * BS : (i + 1) * BS] = res.results[i]["out"].reshape(BS, C, H, W)
    return out
